# revision 1
# baseline (speedup 1.0000x reference)
"""Causal self-attention (GQA, RoPE, QK-RMSNorm) Trainium2 Bass kernel.

Sharding: 8 cores = 2 batches x 4 KV groups. Core i handles batch i//4 and
KV group i%4 (4 query heads + 1 KV head). c_q/c_k/c_v column-sharded,
c_proj row-sharded; the partial output sums are reduced on the host.

Device-side layout trick: the host ships x^T (plus stacked cos/sin tables),
so every matmul contraction dim lands on SBUF partitions with zero on-device
transposes of x. Attention uses the "scores-transposed" formulation:
  s^T[tk, tq] = k^T.T @ q^T  (k block stationary)
  p^T = exp(s^T * scale)     (no max subtraction: QK-RMSNorm bounds |s*scale| <= sqrt(128))
  y[tq, d], r[tq] = p^T.T @ [v | 1]  (ones column gives the softmax denominator)
so softmax needs no partition-dim reductions and no probability transposes.
"""

import sys

sys.path.insert(0, "/opt/trn_rl_repo")

import numpy as np

import concourse.bacc as bacc
import concourse.tile as tile
import concourse.mybir as mybir
from concourse.bass_utils import run_bass_kernel_spmd

# Problem constants (hardcoded per contract)
B = 2
T = 2048
D = 2048
N_HEAD = 16
N_KV = 4
DH = 128
REP = N_HEAD // N_KV  # 4 query heads per KV head
HG = REP * DH  # 512 query columns per core
EPS = 1.1920928955078125e-07
SCALE = 1.0 / float(np.sqrt(DH))
MASK_VAL = -1e9

P = 128
TCH = 512  # T chunk (psum free dim)
NTCH = T // TCH  # 4
NDCH = D // P  # 16
NTB = T // P  # 16 Tk blocks

F32 = mybir.dt.float32
F32R = mybir.dt.float32r
BF16 = mybir.dt.bfloat16

N_CORES = 8

_CACHE = {}


def _build():
    nc = bacc.Bacc("TRN2", num_devices=N_CORES)
    xT = nc.dram_tensor("xT", [D, T], F32, kind="ExternalInput").ap()
    cos2 = nc.dram_tensor("cos2", [P, T], F32, kind="ExternalInput").ap()
    sin2n = nc.dram_tensor("sin2n", [P, T], F32, kind="ExternalInput").ap()
    wq = nc.dram_tensor("wq", [D, HG], F32, kind="ExternalInput").ap()
    wk = nc.dram_tensor("wk", [D, DH], F32, kind="ExternalInput").ap()
    wv = nc.dram_tensor("wv", [D, DH], F32, kind="ExternalInput").ap()
    wo = nc.dram_tensor("wo", [HG, D], F32, kind="ExternalInput").ap()
    out = nc.dram_tensor("out", [T, D], F32, kind="ExternalOutput").ap()

    with tile.TileContext(nc) as tc:
        _trace(tc, xT, cos2, sin2n, wq, wk, wv, wo, out)
    nc.compile()
    return nc


def _trace(tc, xT, cos2, sin2n, wq, wk, wv, wo, out):
    nc = tc.nc
    from contextlib import ExitStack

    ctx = ExitStack()
    with ctx:
        # ---------------- pools ----------------
        const_pool = ctx.enter_context(tc.tile_pool(name="consts", bufs=1))
        attn_pool = ctx.enter_context(tc.tile_pool(name="attn", bufs=1))
        qr_pool = ctx.enter_context(tc.tile_pool(name="qr", bufs=2))
        xt_pool = ctx.enter_context(tc.tile_pool(name="xt", bufs=1))
        wkv_pool = ctx.enter_context(tc.tile_pool(name="wkv", bufs=1))
        wqs_pool = ctx.enter_context(tc.tile_pool(name="wqs", bufs=3))
        tab_pool = ctx.enter_context(tc.tile_pool(name="tab", bufs=1))
        pre_pool = ctx.enter_context(tc.tile_pool(name="pre", bufs=2))
        pt_pool = ctx.enter_context(tc.tile_pool(name="pt", bufs=1))
        yt_pool = ctx.enter_context(tc.tile_pool(name="yt", bufs=1))
        osb_pool = ctx.enter_context(tc.tile_pool(name="osb", bufs=2))
        ps_proj = ctx.enter_context(tc.tile_pool(name="ps_proj", bufs=1, space="PSUM"))
        ps_aux = ctx.enter_context(tc.tile_pool(name="ps_aux", bufs=1, space="PSUM"))
        ps_s = ctx.enter_context(tc.tile_pool(name="ps_s", bufs=2, space="PSUM"))
        ps_o = ctx.enter_context(tc.tile_pool(name="ps_o", bufs=1, space="PSUM"))
        ps_y = ctx.enter_context(tc.tile_pool(name="ps_y", bufs=1, space="PSUM"))
        ps_ytr = ctx.enter_context(tc.tile_pool(name="ps_ytr", bufs=1, space="PSUM"))

        # ---------------- constants ----------------
        ident_f = const_pool.tile([P, P], F32)
        nc.gpsimd.memset(ident_f, 0.0)
        nc.gpsimd.affine_select(
            out=ident_f, in_=ident_f, compare_op=mybir.AluOpType.not_equal,
            fill=1.0, base=0, pattern=[[-1, P]], channel_multiplier=1,
        )
        ident = const_pool.tile([P, P], F32R)
        nc.vector.tensor_copy(out=ident, in_=ident_f)
        # reuse ident_f as the all-ones source once ident is copied out
        nc.vector.memset(ident_f, 1.0)
        onesm = const_pool.tile([P, P], F32R)
        nc.vector.tensor_copy(out=onesm, in_=ident_f)

        # additive causal mask for a diagonal 128x128 block: keep iff col >= row
        mask_sb = const_pool.tile([P, P], F32)
        nc.gpsimd.memset(mask_sb, 0.0)
        nc.gpsimd.affine_select(
            out=mask_sb, in_=mask_sb, compare_op=mybir.AluOpType.is_ge,
            fill=MASK_VAL, base=0, pattern=[[1, P]], channel_multiplier=-1,
        )
        eps_sb = const_pool.tile([P, 1], F32)
        nc.vector.memset(eps_sb, EPS)

        # ---------------- persistent operands ----------------
        krot = attn_pool.tile([P, T], F32R, tag="krot", name="krot")
        vaug = [attn_pool.tile([P, DH + 1], BF16, tag=f"vaug{m}", name=f"vaug{m}")
                for m in range(NTB)]
        wo_sb = attn_pool.tile([P, REP, D], F32R, name="wo_sb")
        wo_r = wo.rearrange("(n p) d -> p n d", p=P).bitcast(F32R)

        wk_sb = wkv_pool.tile([P, NDCH, DH], F32R, name="wk_sb")
        wv_sb = wkv_pool.tile([P, NDCH, DH], F32R, name="wv_sb")
        wk_r = wk.rearrange("(n p) h -> p n h", p=P).bitcast(F32R)
        wv_r = wv.rearrange("(n p) h -> p n h", p=P).bitcast(F32R)
        wq_r = wq.rearrange("(n p) h -> p n h", p=P).bitcast(F32R)
        nc.sync.dma_start(out=wk_sb[:, 0:8, :], in_=wk_r[:, 0:8, :])

        cos2_sb = tab_pool.tile([P, T], F32, name="cos2_sb")
        sin2n_sb = tab_pool.tile([P, T], F32, name="sin2n_sb")

        def load_wq_head(h, j):
            t = wqs_pool.tile([P, NDCH, DH], F32R, tag="wqs", name=f"wqs{h}_{j}")
            hsl = slice(h * DH, (h + 1) * DH)
            nc.sync.dma_start(out=t[:, 0:8, :], in_=wq_r[:, 0:8, hsl])
            nc.sync.dma_start(out=t[:, 8:16, :], in_=wq_r[:, 8:16, hsl])
            return t

        # ---------------- phase A(j): projections + RMS/RoPE + v prep ----------
        # out-major passes over a resident x^T chunk set: 2 psum banks total.
        # pass order (k, v), (q0, q1), (q2, q3): krot first unblocks scores.
        def emit_A_kv(j, qr_tiles):
            cs = slice(j * TCH, (j + 1) * TCH)
            xts = []
            for i in range(NDCH):
                xt = xt_pool.tile([P, TCH], F32R, tag=f"xtc{i}", name=f"xt_{j}_{i}")
                eng = nc.gpsimd if (j == 0 and i % 2 == 1) else nc.sync
                eng.dma_start(
                    out=xt,
                    in_=xT[i * P:(i + 1) * P, j * TCH:(j + 1) * TCH].bitcast(F32R),
                )
                xts.append(xt)
                if j == 0 and i == 1:
                    nc.sync.dma_start(out=wk_sb[:, 8:16, :], in_=wk_r[:, 8:16, :])
                elif j == 0 and i == 4:
                    nc.sync.dma_start(out=wv_sb[:, 0:8, :], in_=wv_r[:, 0:8, :])
                elif j == 0 and i == 5:
                    nc.sync.dma_start(out=wv_sb[:, 8:16, :], in_=wv_r[:, 8:16, :])
                if i == 2:
                    nc.sync.dma_start(out=cos2_sb[:, cs], in_=cos2[:, cs])
                elif i == 3:
                    nc.sync.dma_start(out=sin2n_sb[:, cs], in_=sin2n[:, cs])
                elif j == 0 and i in (6, 9, 12, 15):
                    quarter = {6: 0, 9: 1, 12: 2, 15: 3}[i]
                    nc.sync.dma_start(
                        out=wo_sb[:, quarter, :], in_=wo_r[:, quarter, :])

            def rope(idx, psum, dst):
                # RMS stats: all-ones matmul -> per-column sums on every row
                sq = pre_pool.tile([P, TCH], F32R, tag="sq", name=f"sq{idx}_{j}")
                nc.scalar.activation(
                    out=sq, in_=psum, func=mybir.ActivationFunctionType.Square)
                rps = ps_aux.tile([P, TCH], F32, tag="aux", name=f"rstd{idx}_{j}")
                nc.tensor.matmul(rps, onesm, sq, start=True, stop=True)
                srt = pre_pool.tile([P, TCH], F32, tag="srt", name=f"srt{idx}_{j}")
                nc.scalar.activation(
                    out=srt, in_=rps,
                    func=mybir.ActivationFunctionType.Abs_reciprocal_sqrt,
                    scale=1.0 / DH, bias=eps_sb)
                m1 = pre_pool.tile([P, TCH], F32, tag="m1", name=f"m1_{idx}_{j}")
                nc.vector.tensor_mul(m1, psum, cos2_sb[:, cs])
                # rotate-half: inputs must share a base partition; sin2n ships
                # as [-sin; +sin] so only the output base differs.
                m2 = pre_pool.tile([P, TCH], F32, tag="sq", name=f"m2_{idx}_{j}")
                nc.vector.tensor_mul(m2[0:64, :], psum[64:128, :],
                                     sin2n_sb[64:128, cs])
                nc.vector.tensor_mul(m2[64:128, :], psum[0:64, :],
                                     sin2n_sb[0:64, cs])
                nc.vector.tensor_add(m1, m1, m2)
                nc.vector.tensor_mul(dst, m1, srt)

            def emit_out(idx, lhs_for_i):
                psum = ps_proj.tile([P, TCH], F32, tag="proj", bufs=2,
                                    name=f"proj{idx}_{j}")
                for i in range(NDCH):
                    nc.tensor.matmul(psum, lhs_for_i(i), xts[i],
                                     start=(i == 0), stop=(i == NDCH - 1))
                if idx == 4:  # k
                    rope(idx, psum, krot[:, cs])
                elif idx == 5:  # v: evacuate + transpose blocks + ones col
                    vsb = pre_pool.tile([P, TCH], F32R, tag="vsb", name=f"vsb{j}")
                    nc.scalar.copy(out=vsb, in_=psum)
                    for mm in range(4):
                        m = 4 * j + mm
                        tr = ps_aux.tile([P, P], F32R, tag="aux", name=f"vtr{m}")
                        nc.tensor.transpose(tr, vsb[:, mm * P:(mm + 1) * P], ident)
                        nc.vector.tensor_copy(out=vaug[m][:, 0:DH], in_=tr)
                        nc.vector.memset(vaug[m][:, DH:DH + 1], 1.0)
                else:  # q head
                    rope(idx, psum, qr_tiles[idx])

            # q-slab loads staged between passes (wqs_pool bufs=3)
            wq_tiles = {0: load_wq_head(0, j), 1: load_wq_head(1, j)}
            emit_out(4, lambda i: wk_sb[:, i, :])
            wq_tiles[2] = load_wq_head(2, j)
            emit_out(5, lambda i: wv_sb[:, i, :])
            wq_tiles[3] = load_wq_head(3, j)
            return wq_tiles, emit_out

        # ---------------- phase D(j): attention + output projection ----------
        def emit_D_head(j, qr_tiles, yt, heads):
            for h in heads:
                # double-buffer the low-m tags (reused by every chunk) so
                # exp(h+1) does not wait on pv(h) reads of the same tile
                pts = [pt_pool.tile([P, TCH], BF16, tag=f"pt{m}",
                                    name=f"pt{m}_{j}_{h}",
                                    bufs=(2 if m < 8 else 1))
                       for m in range(4 * j + 4)]
                for m in range(4 * j + 4):
                    sps = ps_s.tile([P, TCH], F32, tag="s", name=f"s{j}_{h}_{m}")
                    nc.tensor.matmul(sps, krot[:, m * P:(m + 1) * P], qr_tiles[h],
                                     start=True, stop=True)
                    if m >= 4 * j:
                        dcol = P * (m - 4 * j)
                        ds_ = slice(dcol, dcol + P)
                        nc.vector.tensor_add(sps[:, ds_], sps[:, ds_], mask_sb)
                        # cols [0:dcol] are never read by any pv matmul
                        nc.scalar.activation(
                            out=pts[m][:, dcol:TCH], in_=sps[:, dcol:TCH],
                            func=mybir.ActivationFunctionType.Exp, scale=SCALE)
                    else:
                        nc.scalar.activation(
                            out=pts[m], in_=sps,
                            func=mybir.ActivationFunctionType.Exp, scale=SCALE)
                def pv_group(n):
                    last = 4 * j + n
                    yps = ps_y.tile([P, DH + 1], F32, tag="y", name=f"y{j}_{h}_{n}")
                    for m in range(last + 1):
                        nc.tensor.matmul(yps, pts[m][:, n * P:(n + 1) * P], vaug[m],
                                         start=(m == 0), stop=(m == last))
                    return yps

                def finish(n, yps):
                    rinv = osb_pool.tile([P, 1], F32, tag="rinv", name=f"ri{j}{h}{n}")
                    nc.vector.reciprocal(out=rinv, in_=yps[:, DH:DH + 1])
                    ynorm = osb_pool.tile([P, P], F32R, tag="ynorm",
                                          name=f"yn{j}{h}{n}")
                    nc.vector.tensor_scalar_mul(ynorm, yps[:, 0:DH], rinv)
                    ytr = ps_ytr.tile([P, P], F32R, tag="ytr", name=f"ytr{j}{h}{n}")
                    nc.tensor.transpose(ytr, ynorm, ident)
                    nc.vector.tensor_copy(out=yt[h][:, n * P:(n + 1) * P], in_=ytr)

                for n in range(4):
                    finish(n, pv_group(n))

        def emit_D_out(j, yt):
            for n in range(4):
                for half in range(2):
                    osb = osb_pool.tile([P, D // 2], F32, tag="osb",
                                        name=f"osb{j}{n}{half}")
                    for dch in range(2):
                        dc = 2 * half + dch
                        ops = ps_o.tile([P, TCH], F32, tag="o", name=f"o{j}_{n}_{dc}")
                        for h in range(REP):
                            nc.tensor.matmul(
                                ops, yt[h][:, n * P:(n + 1) * P],
                                wo_sb[:, h, dc * TCH:(dc + 1) * TCH],
                                start=(h == 0), stop=(h == REP - 1))
                        nc.vector.tensor_copy(
                            out=osb[:, dch * TCH:(dch + 1) * TCH], in_=ops)
                    nc.sync.dma_start(
                        out=out[j * TCH + n * P: j * TCH + (n + 1) * P,
                                half * (D // 2):(half + 1) * (D // 2)],
                        in_=osb)

        # ---------- fine-grained interleave: A(j) passes x D(j-1) heads ------
        qr_all = {}
        yt_all = {}
        for j in range(NTCH):
            qr_all[j] = [
                qr_pool.tile([P, TCH], F32R, tag=f"qr{h}", name=f"qr{h}_{j}")
                for h in range(REP)
            ]
            yt_all[j] = [
                yt_pool.tile([P, TCH], F32R, tag=f"yt{h}", name=f"yt{h}_{j}")
                for h in range(REP)
            ]
            wq_tiles, emit_out = emit_A_kv(j, qr_all[j])
            if j >= 1:
                emit_D_head(j - 1, qr_all[j - 1], yt_all[j - 1], (0, 1))
            emit_out(0, lambda i, t=wq_tiles[0]: t[:, i, :])
            emit_out(1, lambda i, t=wq_tiles[1]: t[:, i, :])
            if j >= 1:
                emit_D_head(j - 1, qr_all[j - 1], yt_all[j - 1], (2, 3))
            emit_out(2, lambda i, t=wq_tiles[2]: t[:, i, :])
            emit_out(3, lambda i, t=wq_tiles[3]: t[:, i, :])
            if j >= 1:
                emit_D_out(j - 1, yt_all[j - 1])
        emit_D_head(NTCH - 1, qr_all[NTCH - 1], yt_all[NTCH - 1], (0, 1, 2, 3))
        emit_D_out(NTCH - 1, yt_all[NTCH - 1])


def _prep_inputs(x, cos, sin, Wq, Wk, Wv, Wo):
    cosT = np.ascontiguousarray(cos[0, :, 0, :].T.astype(np.float32))  # [64, T]
    sinT = np.ascontiguousarray(sin[0, :, 0, :].T.astype(np.float32))
    cos2 = np.concatenate([cosT, cosT], axis=0)
    sin2n = np.concatenate([-sinT, sinT], axis=0)
    in_maps = []
    for i in range(N_CORES):
        b, g = i // 4, i % 4
        in_maps.append({
            "xT": np.ascontiguousarray(x[b].T.astype(np.float32)),
            "cos2": cos2,
            "sin2n": sin2n,
            "wq": np.ascontiguousarray(Wq[:, g * HG:(g + 1) * HG].astype(np.float32)),
            "wk": np.ascontiguousarray(Wk[:, g * DH:(g + 1) * DH].astype(np.float32)),
            "wv": np.ascontiguousarray(Wv[:, g * DH:(g + 1) * DH].astype(np.float32)),
            "wo": np.ascontiguousarray(Wo[g * HG:(g + 1) * HG, :].astype(np.float32)),
        })
    return in_maps


def bench(x, cos, sin, Wq, Wk, Wv, Wo, iters=20):
    """Device-resident timing of the compiled NEFF via the PJRT path.

    Stages all inputs (and fresh donated output buffers) on the devices
    before each timed call, so the measured wall time is dispatch + execute
    + sync only.
    """
    import time

    import jax
    from jax.sharding import Mesh, PartitionSpec
    from jax.experimental.shard_map import shard_map
    import concourse.bass2jax as bass2jax
    import concourse.mybir as mybir_

    if "nc" not in _CACHE:
        _CACHE["nc"] = _build()
    nc = _CACHE["nc"]
    in_maps = _prep_inputs(
        np.asarray(x), np.asarray(cos), np.asarray(sin),
        np.asarray(Wq), np.asarray(Wk), np.asarray(Wv), np.asarray(Wo))

    bass2jax.install_neuronx_cc_hook()
    partition_name = (
        nc.partition_id_tensor.name if nc.partition_id_tensor else None)
    in_names, out_names, out_avals, zero_outs = [], [], [], []
    for alloc in nc.m.functions[0].allocations:
        if not isinstance(alloc, mybir_.MemoryLocationSet):
            continue
        name = alloc.memorylocations[0].name
        if alloc.kind == "ExternalInput":
            if name != partition_name:
                in_names.append(name)
        elif alloc.kind == "ExternalOutput":
            shape = tuple(alloc.tensor_shape)
            dtype = mybir_.dt.np(alloc.dtype)
            out_names.append(name)
            out_avals.append(jax.core.ShapedArray(shape, dtype))
            zero_outs.append(np.zeros(shape, dtype))
    n_params = len(in_names)
    n_outs = len(out_avals)
    all_names = in_names + out_names
    if partition_name is not None:
        all_names = all_names + [partition_name]

    def _body(*args):
        operands = list(args)
        if partition_name is not None:
            operands.append(bass2jax.partition_id_tensor())
        outs = bass2jax._bass_exec_p.bind(
            *operands,
            out_avals=tuple(out_avals),
            in_names=tuple(all_names),
            out_names=tuple(out_names),
            lowering_input_output_aliases=(),
            sim_require_finite=True,
            sim_require_nnan=True,
            nc=nc,
        )
        return tuple(outs)

    devices = jax.devices()[:N_CORES]
    mesh = Mesh(np.asarray(devices), ("core",))
    donate = tuple(range(n_params, n_params + n_outs))
    sharded = jax.jit(
        shard_map(
            _body, mesh=mesh,
            in_specs=(PartitionSpec("core"),) * (n_params + n_outs),
            out_specs=(PartitionSpec("core"),) * n_outs,
            check_rep=False,
        ),
        donate_argnums=donate, keep_unused=True,
    )
    sharding = jax.sharding.NamedSharding(mesh, PartitionSpec("core"))
    concat_in = [
        jax.device_put(
            np.concatenate([np.asarray(in_maps[c][n]) for c in range(N_CORES)], 0),
            sharding)
        for n in in_names
    ]
    jax.block_until_ready(concat_in)

    def fresh_zeros():
        zs = [
            jax.device_put(
                np.zeros((N_CORES * z.shape[0], *z.shape[1:]), z.dtype), sharding)
            for z in zero_outs
        ]
        jax.block_until_ready(zs)
        return zs

    # warmup (compiles the jit)
    outs = sharded(*concat_in, *fresh_zeros())
    jax.block_until_ready(outs)

    times = []
    for _ in range(iters):
        zs = fresh_zeros()
        t0 = time.perf_counter()
        outs = sharded(*concat_in, *zs)
        jax.block_until_ready(outs)
        times.append(time.perf_counter() - t0)
    times = np.array(times)
    return {
        "min_s": float(times.min()),
        "median_s": float(np.median(times)),
        "mean_s": float(times.mean()),
        "all_s": times.tolist(),
    }


def kernel(x, cos, sin, Wq, Wk, Wv, Wo, _trace_flag=False):
    if "nc" not in _CACHE:
        _CACHE["nc"] = _build()
    nc = _CACHE["nc"]
    in_maps = _prep_inputs(
        np.asarray(x), np.asarray(cos), np.asarray(sin),
        np.asarray(Wq), np.asarray(Wk), np.asarray(Wv), np.asarray(Wo))
    res = run_bass_kernel_spmd(
        nc, in_maps, core_ids=list(range(N_CORES)), trace=_trace_flag)
    _CACHE["last_result"] = res
    out = np.empty((B, T, D), dtype=np.float32)
    for b in range(B):
        acc = res.results[4 * b]["out"].astype(np.float32).copy()
        for g in range(1, 4):
            acc += res.results[4 * b + g]["out"]
        out[b] = acc
    return out

